# revision 22
# baseline (speedup 1.0000x reference)
"""Trainium2 Bass kernel: 16-head MHA with RoPE (B=2, L=4096, H=1024, HD=64).

Sharding (8 cores): core c -> batch b = c//4, head group g = c%4 (4 heads each,
as 2 head-pairs).  Each core computes Q/K/V projections for its heads only,
RoPE, scoresT = K'q'^T in [k, q] layout (row-packed 2-heads-per-PE-pass,
contraction d=64 at tile rows 0/64), exp on ScalarE straight out of PSUM,
PV with a fused ones-column for the softmax denominator, normalization via a
DMA partition-broadcast of 1/Z, then an AllToAll so every core ends up with
attn'T[all 1024 hd-dims, its 1024 query rows] and computes exact final output
rows through the O projection.  All matmuls run in bf16 (fp32 PSUM accum).

The attention_mask input is structurally zero (spec fill=zeros); the device
kernel skips it and the host wrapper falls back to a numpy reference in the
(never exercised) case of a non-zero mask.
"""

import sys

for _p in ("/opt/trn_rl_repo", "/root/.axon_site/_ro/trn_rl_repo"):
    if _p not in sys.path:
        sys.path.insert(0, _p)

import numpy as np
import ml_dtypes

import concourse.bass as bass
import concourse.mybir as mybir
import concourse.tile as tile
from concourse import bacc
from concourse.bass_utils import run_bass_kernel_spmd
from concourse.masks import make_identity

F32 = mybir.dt.float32
BF16 = mybir.dt.bfloat16

H = 1024
NH = 16
HD = 64
B = 2
NCORES = 8
GROUPS = NCORES // B          # 4 cores share one batch element
HPC = NH // GROUPS            # 4 heads per core
NHP = HPC // 2                # 2 head-pairs per core
BASE = 10000.0

_PROG_CACHE = {}


def _kc_groups(n_kc, grp=3):
    out, i = [], 0
    while i < n_kc:
        out.append(list(range(i, min(i + grp, n_kc))))
        i += grp
    return out


def build_program(L=4096, use_collective=True):
    """One SPMD program; per-core behavior differs only through inputs."""
    nc = bacc.Bacc("TRN2", target_bir_lowering=False, debug=False,
                   num_devices=NCORES)

    n_tch = L // 128        # 128-row chunks of the sequence
    n_tsl = L // 512        # 512-col slices of the sequence
    n_qb = L // 512         # query blocks
    n_kc = L // 128         # key chunks
    LQ = L // GROUPS        # query rows owned by this core in phase C

    hid_d = nc.declare_dram_parameter("hid", [L, H], F32, isOutput=False)
    wq_d = nc.declare_dram_parameter("wq", [128, 8 * 512], BF16, isOutput=False)
    wk_d = nc.declare_dram_parameter("wk", [128, 8 * 512], BF16, isOutput=False)
    wv_d = nc.declare_dram_parameter("wv", [128, 8 * 256], BF16, isOutput=False)
    if use_collective:
        wo_d = nc.declare_dram_parameter("wo", [128, 16 * 1024], BF16,
                                         isOutput=False)
    cos_d = nc.declare_dram_parameter("cos", [128, L], BF16, isOutput=False)
    sin_d = nc.declare_dram_parameter("sin", [128, L], BF16, isOutput=False)
    ident_d = nc.declare_dram_parameter("ident", [128, 128], F32, isOutput=False)
    if use_collective:
        out_d = nc.declare_dram_parameter("out", [LQ, H], F32, isOutput=True)
    else:
        out_d = nc.declare_dram_parameter("attn_out", [HPC * 64, L], BF16,
                                          isOutput=True)

    with tile.TileContext(nc) as tc:
        with tc.tile_pool(name="persist", bufs=1) as persist:
            ident = persist.tile([128, 128], F32, name="ident", tag="ident")
            nc.sync.dma_start(out=ident[:], in_=ident_d[:])
            ones64 = persist.tile([1, 64], F32, name="ones64", tag="ones64")
            nc.vector.memset(ones64[:], 1.0)

            wq_sb = persist.tile([128, 8 * 512], BF16, name="wq", tag="wq")
            wk_sb = persist.tile([128, 8 * 512], BF16, name="wk", tag="wk")
            wv_sb = persist.tile([128, 8 * 256], BF16, name="wv", tag="wv")
            cos_sb = persist.tile([128, L], BF16, name="cos", tag="cos")
            sin_sb = persist.tile([128, L], BF16, name="sin", tag="sin")
            loads = [(wq_sb, wq_d), (wk_sb, wk_d), (wv_sb, wv_d),
                     (cos_sb, cos_d), (sin_sb, sin_d)]
            for sb, d in loads:
                nc.sync.dma_start(out=sb[:], in_=d[:])

            qt = [persist.tile([128, L], BF16, name=f"qt{i}", tag=f"qt{i}") for i in range(NHP)]
            kt = [persist.tile([128, L], BF16, name=f"kt{i}", tag=f"kt{i}") for i in range(NHP)]
            vsb = [persist.tile([128, HPC * 68 + 64], BF16, name=f"v{i}", tag=f"v{i}")
                   for i in range(n_tch)]
            attn = [persist.tile([128, L], BF16, name=f"at{i}", tag=f"at{i}") for i in range(NHP)]

            for tch in range(n_tch):
                nc.vector.memset(vsb[tch][:], 0.0)
                for hl in range(HPC):
                    nc.vector.memset(vsb[tch][:, hl * 68 + 64: hl * 68 + 65], 1.0)

            # ---- Phase A: load hidden, transpose to hidT (bf16) --------------
            with tc.tile_pool(name="hidT", bufs=1) as hidT_pool:
                hidT = [hidT_pool.tile([128, L], BF16, name=f"hT{i}", tag=f"hT{i}")
                        for i in range(8)]
                with (tc.tile_pool(name="hin", bufs=3) as hin_pool,
                      tc.tile_pool(name="tp_ps", bufs=4, space="PSUM") as tp_ps):
                    for tch in range(n_tch):
                        hrow = hin_pool.tile([128, H], F32, name="hin", tag="hin")
                        nc.sync.dma_start(out=hrow[:],
                                          in_=hid_d[tch * 128:(tch + 1) * 128, :])
                        for c in range(8):
                            tp = tp_ps.tile([128, 128], F32, name="tp", tag="tp")
                            nc.tensor.transpose(
                                tp[:], hrow[:, c * 128:(c + 1) * 128], ident[:])
                            nc.vector.tensor_copy(
                                hidT[c][:, tch * 128:(tch + 1) * 128], tp[:])

                # ---- Phase A2: projections + RoPE ---------------------------
                with (tc.tile_pool(name="pj_ps", bufs=4, space="PSUM") as pj_ps,
                      tc.tile_pool(name="rope_t", bufs=4) as rope_t):
                    for hp in range(NHP):
                        for w_sb, dest in ((wq_sb, qt[hp]), (wk_sb, kt[hp])):
                            for tsl in range(n_tsl):
                                t0 = tsl * 512
                                pl = pj_ps.tile([128, 512], F32, name="pj", tag="pj")
                                ro = pj_ps.tile([128, 512], F32, name="pj", tag="pj")
                                for c in range(8):
                                    lhs_p = w_sb[:, c * 512 + hp * 128:
                                                 c * 512 + hp * 128 + 128]
                                    lhs_r = w_sb[:, c * 512 + 256 + hp * 128:
                                                 c * 512 + 256 + hp * 128 + 128]
                                    rhs = hidT[c][:, t0:t0 + 512]
                                    nc.tensor.matmul(pl[:], lhs_p, rhs,
                                                     start=(c == 0), stop=(c == 7))
                                    nc.tensor.matmul(ro[:], lhs_r, rhs,
                                                     start=(c == 0), stop=(c == 7))
                                t1 = rope_t.tile([128, 512], BF16, name="r1", tag="r1")
                                t2 = rope_t.tile([128, 512], BF16, name="r2", tag="r2")
                                nc.vector.tensor_mul(t1[:], ro[:],
                                                     sin_sb[:, t0:t0 + 512])
                                nc.vector.tensor_mul(t2[:], pl[:],
                                                     cos_sb[:, t0:t0 + 512])
                                nc.vector.tensor_add(dest[:, t0:t0 + 512],
                                                     t1[:], t2[:])
                    # V projection (natural layout, per-head 68-col strips)
                    for tch in range(n_tch):
                        vp = pj_ps.tile([128, HPC * 64], F32, name="pj", tag="pj")
                        for c in range(8):
                            nc.tensor.matmul(
                                vp[:], hidT[c][:, tch * 128:(tch + 1) * 128],
                                wv_sb[:, c * 256:(c + 1) * 256],
                                start=(c == 0), stop=(c == 7))
                        for hl in range(HPC):
                            nc.vector.tensor_copy(
                                vsb[tch][:, hl * 68: hl * 68 + 64],
                                vp[:, hl * 64:(hl + 1) * 64])

            # ---- Phase B: attention ----------------------------------------
            groups = _kc_groups(n_kc, 2)
            with (tc.tile_pool(name="sc_ps", bufs=2, space="PSUM") as sc_ps,
                  tc.tile_pool(name="pv_ps", bufs=2, space="PSUM") as pv_ps,
                  tc.tile_pool(name="probs", bufs=4) as probs_pool,
                  tc.tile_pool(name="zdram", bufs=4, space="DRAM") as zdram,
                  tc.tile_pool(name="nrm", bufs=4) as nrm_pool):
                for hp in range(NHP):
                    for qb in range(n_qb):
                        q0 = qb * 512
                        pvs = [pv_ps.tile([128, 512], F32, name=f"pv{h}", tag=f"pv{h}")
                               for h in range(2)]
                        for kcs in groups:
                            scs, prs = [], []
                            for h in range(2):
                                scs.append(sc_ps.tile([128, 512 * len(kcs)],
                                                      F32, name="sc", tag="sc"))
                                prs.append(probs_pool.tile(
                                    [128, 512 * len(kcs)], BF16, name="pr", tag="pr"))
                            for ki, kc in enumerate(kcs):
                                for h in range(2):
                                    nc.tensor.matmul(
                                        scs[h][:, ki * 512:(ki + 1) * 512],
                                        kt[hp][h * 64:(h + 1) * 64,
                                               kc * 128:(kc + 1) * 128],
                                        qt[hp][h * 64:(h + 1) * 64, q0:q0 + 512],
                                        start=True, stop=True,
                                        tile_position=(h * 64, 0))
                            for h in range(2):
                                nc.scalar.activation(
                                    prs[h][:], scs[h][:],
                                    mybir.ActivationFunctionType.Exp,
                                    scale=float(1.0 / np.sqrt(HD)))
                            for ki, kc in enumerate(kcs):
                                for h in range(2):
                                    hl = 2 * hp + h
                                    nc.tensor.matmul(
                                        pvs[h][:],
                                        vsb[kc][:, hl * 68: hl * 68 + 128],
                                        prs[h][:, ki * 512:(ki + 1) * 512],
                                        start=(kc == 0), stop=(kc == n_kc - 1))
                        for h in range(2):
                            recip = nrm_pool.tile([1, 512], F32, name="rc", tag="rc")
                            zdr = zdram.tile([1, 512], F32, name="zd", tag="zd")
                            bcast = nrm_pool.tile([64, 512], F32, name="bc", tag="bc")
                            nc.vector.reciprocal(recip[:], pvs[h][64:65, :])
                            nc.sync.dma_start(out=zdr[:], in_=recip[:])
                            zsrc = zdr[:]
                            nc.sync.dma_start(
                                out=bcast[:],
                                in_=bass.AP(tensor=zsrc.tensor, offset=zsrc.offset,
                                            ap=[[0, 64]] + list(zsrc.ap)[1:]))
                            nc.vector.tensor_mul(
                                attn[hp][h * 64:(h + 1) * 64, q0:q0 + 512],
                                pvs[h][0:64, :], bcast[:])

            # ---- Phase C: exchange + O projection --------------------------
            if use_collective:
                LQ_ = LQ
                with (tc.tile_pool(name="dram", bufs=1, space="DRAM") as dram,
                      tc.tile_pool(name="osb", bufs=3) as osb_pool,
                      tc.tile_pool(name="wop", bufs=1) as wop_pool,
                      tc.tile_pool(name="o_ps", bufs=4, space="PSUM") as o_ps):
                    wo_sb = wop_pool.tile([128, 16 * 1024], BF16,
                                          name="wo", tag="wo")
                    nc.sync.dma_start(out=wo_sb[:], in_=wo_d[:])
                    # 8-way AllToAll: chunk j = my heads x (j%4)-th query
                    # quarter.  Cross-quad blocks are junk at the receiver and
                    # get zeroed by the per-core-masked wo input.
                    ag_in = dram.tile([2 * H, LQ_], BF16, name="ag_in")
                    ag_out = dram.tile([2 * H, LQ_], BF16, name="ag_out")
                    for j in range(NCORES):
                        for hp in range(NHP):
                            nc.sync.dma_start(
                                out=ag_in[j * 256 + hp * 128:
                                          j * 256 + hp * 128 + 128, :],
                                in_=attn[hp][:, (j % GROUPS) * LQ_:
                                             (j % GROUPS + 1) * LQ_])
                    nc.gpsimd.collective_compute(
                        "AllToAll", mybir.AluOpType.bypass,
                        replica_groups=[list(range(NCORES))],
                        ins=[ag_in.opt()], outs=[ag_out.opt()])
                    for tch in range(LQ_ // 128):
                        ops = [o_ps.tile([128, 512], F32, name="op", tag="op")
                               for _ in range(2)]
                        for c2 in range(16):
                            lt = osb_pool.tile([128, 128], BF16, name="lt", tag="lt")
                            nc.sync.dma_start(
                                out=lt[:],
                                in_=ag_out[c2 * 128:(c2 + 1) * 128,
                                           tch * 128:(tch + 1) * 128])
                            for nhalf in range(2):
                                nc.tensor.matmul(
                                    ops[nhalf][:], lt[:],
                                    wo_sb[:, c2 * 1024 + nhalf * 512:
                                          c2 * 1024 + nhalf * 512 + 512],
                                    start=(c2 == 0), stop=(c2 == 15))
                        orow = osb_pool.tile([128, H], F32, name="or", tag="or")
                        for nhalf in range(2):
                            nc.vector.tensor_copy(
                                orow[:, nhalf * 512:(nhalf + 1) * 512],
                                ops[nhalf][:])
                        nc.sync.dma_start(
                            out=out_d[tch * 128:(tch + 1) * 128, :], in_=orow[:])
            else:
                for hp in range(NHP):
                    nc.sync.dma_start(
                        out=out_d[hp * 128:(hp + 1) * 128, :], in_=attn[hp][:])
    nc.finalize()
    return nc


def build_oproj_program(L=4096):
    """Fallback phase-C program when in-NEFF collectives are unavailable."""
    nc = bacc.Bacc("TRN2", target_bir_lowering=False, debug=False,
                   num_devices=NCORES)
    LQ = L // GROUPS
    agm_d = nc.declare_dram_parameter("agm", [H, LQ], BF16, isOutput=False)
    wo_d = nc.declare_dram_parameter("wo", [128, 8 * 1024], BF16, isOutput=False)
    out_d = nc.declare_dram_parameter("out", [LQ, H], F32, isOutput=True)
    with tile.TileContext(nc) as tc:
        with (tc.tile_pool(name="persist", bufs=1) as persist,
              tc.tile_pool(name="osb", bufs=3) as osb_pool,
              tc.tile_pool(name="o_ps", bufs=4, space="PSUM") as o_ps):
            wo_sb = persist.tile([128, 8 * 1024], BF16, name="wo", tag="wo")
            nc.sync.dma_start(out=wo_sb[:], in_=wo_d[:])
            for tch in range(LQ // 128):
                ops = [o_ps.tile([128, 512], F32, name="op", tag="op") for _ in range(2)]
                for c2 in range(8):
                    lt = osb_pool.tile([128, 128], BF16, name="lt", tag="lt")
                    nc.sync.dma_start(
                        out=lt[:], in_=agm_d[c2 * 128:(c2 + 1) * 128,
                                             tch * 128:(tch + 1) * 128])
                    for nhalf in range(2):
                        nc.tensor.matmul(
                            ops[nhalf][:], lt[:],
                            wo_sb[:, c2 * 1024 + nhalf * 512:
                                  c2 * 1024 + nhalf * 512 + 512],
                            start=(c2 == 0), stop=(c2 == 7))
                orow = osb_pool.tile([128, H], F32, name="or", tag="or")
                for nhalf in range(2):
                    nc.vector.tensor_copy(
                        orow[:, nhalf * 512:(nhalf + 1) * 512], ops[nhalf][:])
                nc.sync.dma_start(out=out_d[tch * 128:(tch + 1) * 128, :],
                                  in_=orow[:])
    nc.finalize()
    return nc


# ---------------------------------------------------------------------------
# Host-side input preparation
# ---------------------------------------------------------------------------

def _rope_tables(L):
    inv_freq = 1.0 / (BASE ** (np.arange(0, HD, 2, dtype=np.float32) / HD))
    t = np.arange(L, dtype=np.float32)
    freqs = np.outer(t, inv_freq)                       # [L, 32]
    emb = np.concatenate([freqs, freqs], axis=-1)       # [L, 64]
    cosT = np.cos(emb).T.astype(ml_dtypes.bfloat16)     # [64, L]
    sinT = np.sin(emb).T.astype(ml_dtypes.bfloat16)
    return (np.concatenate([cosT, cosT], axis=0),
            np.concatenate([sinT, sinT], axis=0))       # [128, L]


def _pack_chunked(wT, blk):
    """[1024, blk] -> [128, 8*blk] with [p, c*blk+j] = wT[c*128+p, j]."""
    return np.ascontiguousarray(
        wT.reshape(8, 128, blk).transpose(1, 0, 2).reshape(128, 8 * blk))


def _rot_cols(wT_c):
    """Per-64-col head block: col d<32 -> -col(d+32); col d>=32 -> col(d-32)."""
    out = np.empty_like(wT_c)
    nheads = wT_c.shape[1] // HD
    for h in range(nheads):
        blkc = wT_c[:, h * HD:(h + 1) * HD]
        out[:, h * HD: h * HD + 32] = -blkc[:, 32:64]
        out[:, h * HD + 32: (h + 1) * HD] = blkc[:, 0:32]
    return out


def make_in_maps(hidden_states, Wq, Wk, Wv, Wo, L, use_collective=True):
    bf = ml_dtypes.bfloat16
    cos_pk, sin_pk = _rope_tables(L)
    WoT = np.ascontiguousarray(Wo.T).astype(bf)            # [1024, 1024]
    zeros = np.zeros_like(WoT)
    if use_collective:
        # [2048, 1024] with the real WoT in this core's quad block
        WoT_q = {0: np.concatenate([WoT, zeros], axis=0),
                 1: np.concatenate([zeros, WoT], axis=0)}
        wo_pk = {q: np.ascontiguousarray(
            WoT_q[q].reshape(16, 128, 1024).transpose(1, 0, 2)
            .reshape(128, 16 * 1024)) for q in (0, 1)}
    in_maps = []
    for c in range(NCORES):
        b, g = c // GROUPS, c % GROUPS
        hs = g * HPC * HD
        maps = {"hid": np.ascontiguousarray(hidden_states[b]).astype(np.float32),
                "cos": cos_pk, "sin": sin_pk,
                "ident": np.eye(128, dtype=np.float32)}
        if use_collective:
            maps["wo"] = wo_pk[b]
        for nm, W in (("wq", Wq), ("wk", Wk)):
            WT_c = np.ascontiguousarray(W[hs:hs + HPC * HD, :].T)  # [1024, 256]
            pl = WT_c.reshape(8, 128, 256)
            ro = _rot_cols(WT_c).reshape(8, 128, 256)
            maps[nm] = np.ascontiguousarray(
                np.concatenate([pl, ro], axis=2)      # [8, 128, 512]
                .transpose(1, 0, 2).reshape(128, 8 * 512)).astype(bf)
        maps["wv"] = _pack_chunked(
            np.ascontiguousarray(Wv[hs:hs + HPC * HD, :].T).astype(bf), 256)
        in_maps.append(maps)
    return in_maps


def _numpy_reference(hidden_states, attention_mask, Wq, Wk, Wv, Wo):
    b, l, _ = hidden_states.shape
    x = hidden_states.astype(np.float64)
    q = (x @ Wq.T.astype(np.float64)).reshape(b, l, NH, HD).transpose(0, 2, 1, 3)
    k = (x @ Wk.T.astype(np.float64)).reshape(b, l, NH, HD).transpose(0, 2, 1, 3)
    v = (x @ Wv.T.astype(np.float64)).reshape(b, l, NH, HD).transpose(0, 2, 1, 3)
    inv_freq = 1.0 / (BASE ** (np.arange(0, HD, 2) / HD))
    t = np.arange(l)
    emb = np.concatenate([np.outer(t, inv_freq)] * 2, axis=-1)
    cos, sin = np.cos(emb)[None, None], np.sin(emb)[None, None]

    def rot(z):
        z1, z2 = z[..., :HD // 2], z[..., HD // 2:]
        return np.concatenate([-z2, z1], axis=-1)

    q = q * cos + rot(q) * sin
    k = k * cos + rot(k) * sin
    s = np.einsum("bhqd,bhkd->bhqk", q, k) / np.sqrt(HD)
    s = s + attention_mask.astype(np.float64)
    s -= s.max(axis=-1, keepdims=True)
    p = np.exp(s)
    p /= p.sum(axis=-1, keepdims=True)
    o = np.einsum("bhqk,bhkd->bhqd", p, v)
    o = o.transpose(0, 2, 1, 3).reshape(b, l, H)
    return (o @ Wo.T.astype(np.float64)).astype(np.float32)


def run_on_hw(inputs, L=4096, use_collective=True, trace=False):
    """Returns (out [B, L, H] f32, BassKernelResults)."""
    key = (L, use_collective)
    if key not in _PROG_CACHE:
        _PROG_CACHE[key] = build_program(L=L, use_collective=use_collective)
    nc = _PROG_CACHE[key]
    in_maps = make_in_maps(inputs["hidden_states"], inputs["Wq"], inputs["Wk"],
                           inputs["Wv"], inputs["Wo"], L, use_collective)
    br = run_bass_kernel_spmd(nc, in_maps, list(range(NCORES)), trace=trace)
    LQ = L // GROUPS
    out = np.empty((B, L, H), np.float32)
    if use_collective:
        for c in range(NCORES):
            b, g = c // GROUPS, c % GROUPS
            out[b, g * LQ:(g + 1) * LQ, :] = br.results[c]["out"]
    else:
        # host exchange + second program for the O projection
        attnT = {b: np.empty((H, L), ml_dtypes.bfloat16) for b in range(B)}
        for c in range(NCORES):
            b, g = c // GROUPS, c % GROUPS
            attnT[b][g * 256:(g + 1) * 256, :] = br.results[c]["attn_out"]
        key2 = ("oproj", L)
        if key2 not in _PROG_CACHE:
            _PROG_CACHE[key2] = build_oproj_program(L=L)
        bf = ml_dtypes.bfloat16
        WoT = _pack_chunked(np.ascontiguousarray(inputs["Wo"].T).astype(bf), 1024)
        in_maps2 = []
        for c in range(NCORES):
            b, g = c // GROUPS, c % GROUPS
            in_maps2.append({
                "agm": np.ascontiguousarray(attnT[b][:, g * LQ:(g + 1) * LQ]),
                "wo": WoT})
        br2 = run_bass_kernel_spmd(_PROG_CACHE[key2], in_maps2,
                                   list(range(NCORES)), trace=trace)
        for c in range(NCORES):
            b, g = c // GROUPS, c % GROUPS
            out[b, g * LQ:(g + 1) * LQ, :] = br2.results[c]["out"]
        if br.exec_time_ns and br2.exec_time_ns:
            br.exec_time_ns += br2.exec_time_ns
    return out, br


def kernel(hidden_states, attention_mask, Wq, Wk, Wv, Wo):
    hidden_states = np.asarray(hidden_states)
    attention_mask = np.asarray(attention_mask)
    if attention_mask.size and np.any(attention_mask):
        return _numpy_reference(hidden_states, attention_mask,
                                np.asarray(Wq), np.asarray(Wk),
                                np.asarray(Wv), np.asarray(Wo))
    inputs = {"hidden_states": hidden_states, "Wq": np.asarray(Wq),
              "Wk": np.asarray(Wk), "Wv": np.asarray(Wv),
              "Wo": np.asarray(Wo)}
    out, _ = run_on_hw(inputs, L=hidden_states.shape[1], use_collective=True)
    return out
